# revision 14
# baseline (speedup 1.0000x reference)
"""GCN (2-layer, PyG GCNConv-style) Trainium2 Bass kernel, 8-core SPMD.

Strategy:
  - Pad nodes to NPAD = 8*49*128 = 50176. Core c owns destination nodes
    [c*6272, (c+1)*6272) = 49 blocks of 128.
  - Append self-loops, sort edges by (dst_block, src). Within each dst
    block, edges are split into "lo" (src < 25088) and "hi" (src >= 25088)
    groups so gather indices fit in int16 (dma_gather requirement), each
    group padded to a fixed chunk count (CLO/CHI chunks of 128 edges)
    common to all cores (SPMD: one program, per-core data).
  - GCN symmetric normalization is separable: norm[e] = dinv[src]*dinv[dst]
    is folded into the per-chunk selection matrix
        S[e, d] = norm[e] * (dst_rel[e] == d)
    built with a single DVE tensor_scalar(is_equal, mult) against a
    constant iota tile.
  - Aggregation commutes with the weight matmul: A@(X@W) = (A@X)@W, so we
    gather RAW node features (fp16) and apply W per 128-dst block:
        BT[f, d] += G_chunk[e, f].T @ S_chunk[e, d]     (PSUM accumulate)
        H[d, :]   = relu(BT.T @ W + b)
  - Per-edge feature traffic runs through batched dma_gather (256-byte fp16
    rows near the DMA descriptor floor).
  - Two NEFF launches (one per GCN layer): device collectives are broken
    under this runtime, so layer-1 output shards are gathered on the host
    and fed to launch 2 as the (replicated) gather table.
"""

import sys

sys.path.insert(0, "/opt/trn_rl_repo")

import numpy as np

import concourse.bacc as bacc
import concourse.mybir as mybir
import concourse.tile as tile
from concourse.bass_utils import run_bass_kernel_spmd

# ---------------------------------------------------------------- constants
N = 50000
F0, F1, F2 = 64, 128, 64
NC = 8          # cores
P = 128         # partitions / dst-block size / edge-chunk size
BPC = 49        # dst blocks per core
NPC = BPC * P   # 6272 nodes per core
NPAD = NC * NPC  # 50176
NBLK = NC * BPC  # 392
HALF = NPAD // 2  # 25088, int16-safe table split point
FT = 128        # feature width of both gather tables (256B fp16 rows)
GMAX = 8        # max chunks (x128 idxs) per dma_gather: SWDGE ring holds 1024 descs

_cache = {}


# ---------------------------------------------------------------- builder
def _build(CLO, CHI, fout, out_f32):
    """One GCN layer: gather from xtab, aggregate per dst block, apply W+b,
    relu. fout: output feature count. out_f32: fp32 output (final layer)
    vs fp16 (intermediate, feeds the next layer's gather table)."""
    C = CLO + CHI
    dt = mybir.dt
    odt = dt.float32 if out_f32 else dt.float16
    nc = bacc.Bacc("TRN2", target_bir_lowering=False, debug=False, num_devices=NC)

    xtab = nc.dram_tensor("xtab", [NPAD, FT], dt.float16, kind="ExternalInput").ap()
    eidx = nc.dram_tensor("eidx", [P, BPC * C * 8], dt.int16, kind="ExternalInput").ap()
    edst = nc.dram_tensor("edst", [P, BPC * C], dt.float32, kind="ExternalInput").ap()
    enrm = nc.dram_tensor("enrm", [P, BPC * C], dt.float32, kind="ExternalInput").ap()
    w = nc.dram_tensor("w", [FT, fout], dt.float16, kind="ExternalInput").ap()
    bb = nc.dram_tensor("bb", [P, fout], dt.float32, kind="ExternalInput").ap()
    iot = nc.dram_tensor("iot", [P, P], dt.float16, kind="ExternalInput").ap()
    cnt = nc.dram_tensor("cnt", [P, BPC * 2], dt.int32, kind="ExternalInput").ap()
    out = nc.dram_tensor("out", [NPC, fout], odt, kind="ExternalOutput").ap()

    Alu = mybir.AluOpType

    with (
        tile.TileContext(nc) as tc,
        tc.tile_pool(name="res", bufs=1) as res,
    ):
        def resident(name, shape, dtype, src_ap):
            t = res.tile(shape, dtype, name=name, tag=name)
            nc.sync.dma_start(t[:], src_ap)
            return t

        eidx_sb = resident("eidx_sb", [P, BPC * C * 8], dt.int16, eidx)
        edst_sb = resident("edst_sb", [P, BPC * C], dt.float32, edst)
        enrm_sb = resident("enrm_sb", [P, BPC * C], dt.float32, enrm)
        w_sb = resident("w_sb", [FT, fout], dt.float16, w)
        bb_sb = resident("bb_sb", [P, fout], dt.float32, bb)
        iot_sb = resident("iot_sb", [P, P], dt.float16, iot)
        cnt_sb = resident("cnt_sb", [P, BPC * 2], dt.int32, cnt)

        stage = res.tile([P, BPC, fout], odt, name="stage", tag="stage")

        # Explicit rotating gather buffers: padded (invalid, idx=-1) tail rows
        # are skipped by the DMA and keep stale data, so the buffers must
        # start finite (0 * S_pad = 0, not NaN).
        NGT = 3
        gts = []
        for i in range(NGT):
            g = res.tile([P, C, FT], dt.float16, name=f"gt{i}", tag=f"gt{i}")
            nc.vector.memset(g[:], 0.0)
            gts.append(g)
        rlo = nc.alloc_registers("rlo", engines=[mybir.EngineType.Pool])[mybir.EngineType.Pool]
        rhi = nc.alloc_registers("rhi", engines=[mybir.EngineType.Pool])[mybir.EngineType.Pool]

        with (
            tc.tile_pool(name="sp", bufs=4) as sp,
            tc.tile_pool(name="btp", bufs=2, space="PSUM") as btp,
            tc.tile_pool(name="hp", bufs=2, space="PSUM") as hp,
            tc.tile_pool(name="sbx", bufs=3) as sbx,
        ):
            for b in range(BPC):
                gt = gts[b % NGT]
                ic = b * C * 8
                # single_packet=False streams descriptors through the SWDGE
                # ring with flow control, so one instruction can exceed the
                # 1024-descriptor ring capacity: one gather per table half.
                # Valid-index counts come from per-core data via Pool
                # registers; the trailing -1 pads generate no descriptors.
                segs = [
                    (0, CLO, xtab[0:HALF, :], rlo, 0),
                    (CLO, CHI, xtab[HALF:NPAD, :], rhi, 1),
                ]
                for c0, nch, table, reg, half in segs:
                    nc.gpsimd.reg_load(reg, cnt_sb[0:1, 2 * b + half : 2 * b + half + 1])
                    nc.gpsimd.dma_gather(
                        out_ap=gt[:, c0 : c0 + nch, :],
                        in_ap=table,
                        idxs_ap=eidx_sb[:, ic + c0 * 8 : ic + (c0 + nch) * 8],
                        num_idxs=nch * P,
                        num_idxs_reg=reg,
                        elem_size=FT,
                        single_packet=False,
                    )
                bt = btp.tile([FT, P], dt.float32, tag="bt")
                for c in range(C):
                    k = b * C + c
                    s = sp.tile([P, P], dt.float16, tag="s")
                    nc.vector.tensor_scalar(
                        out=s[:],
                        in0=iot_sb[:],
                        scalar1=edst_sb[:, k : k + 1],
                        scalar2=enrm_sb[:, k : k + 1],
                        op0=Alu.is_equal,
                        op1=Alu.mult,
                    )
                    nc.tensor.matmul(
                        out=bt[:],
                        lhsT=gt[:, c, :],
                        rhs=s[:],
                        start=(c == 0),
                        stop=(c == C - 1),
                    )
                btsb = sbx.tile([FT, P], dt.float16, tag="btsb")
                nc.vector.tensor_copy(out=btsb[:], in_=bt[:])
                h = hp.tile([P, fout], dt.float32, tag="h")
                nc.tensor.matmul(
                    out=h[:], lhsT=btsb[:], rhs=w_sb[:], start=True, stop=True
                )
                t = sbx.tile([P, fout], dt.float32, tag="t")
                nc.vector.tensor_tensor(out=t[:], in0=h[:], in1=bb_sb[:], op=Alu.add)
                nc.vector.tensor_scalar(
                    out=stage[:, b, :], in0=t[:], scalar1=0.0, scalar2=None,
                    op0=Alu.max,
                )

        # node n = b*128+p  ->  row-major [NPC, fout]
        nc.sync.dma_start(
            out=out[:].rearrange("(b p) f -> p b f", p=P),
            in_=stage[:],
        )

    nc.compile()
    return nc


# ---------------------------------------------------------------- host prep
def _preprocess(z, edge_index, W1, b1, W2, b2):
    src = np.asarray(edge_index[0], dtype=np.int64)
    dst = np.asarray(edge_index[1], dtype=np.int64)
    loops = np.arange(N, dtype=np.int64)
    src = np.concatenate([src, loops])
    dst = np.concatenate([dst, loops])

    deg = np.bincount(dst, minlength=NPAD).astype(np.float32)
    dinv = np.zeros(NPAD, dtype=np.float32)
    nz = deg > 0
    dinv[nz] = 1.0 / np.sqrt(deg[nz])
    norm = (dinv[src] * dinv[dst]).astype(np.float32)

    blk = (dst >> 7).astype(np.int64)
    order = np.lexsort((src, blk))
    src_s, dst_s, nrm_s, blk_s = src[order], dst[order], norm[order], blk[order]
    is_hi = src_s >= HALF

    cnt = np.bincount(blk_s, minlength=NBLK)
    cnt_lo = np.bincount(blk_s[~is_hi], minlength=NBLK)
    CLO = int(-(-cnt_lo.max() // P))
    CHI = int(-(-(cnt - cnt_lo).max() // P))
    C = CLO + CHI

    blk_start = np.zeros(NBLK, dtype=np.int64)
    np.cumsum(cnt[:-1], out=blk_start[1:])
    pos_in_blk = np.arange(len(src_s)) - blk_start[blk_s]
    slot = np.where(~is_hi, pos_in_blk, CLO * P + (pos_in_blk - cnt_lo[blk_s]))
    col = blk_s * (C * P) + slot

    idx_flat = np.full(NBLK * C * P, -1, dtype=np.int16)
    idx_flat[col] = np.where(is_hi, src_s - HALF, src_s).astype(np.int16)
    dst_flat = np.full(NBLK * C * P, -1.0, dtype=np.float32)
    dst_flat[col] = (dst_s & 127).astype(np.float32)
    nrm_flat = np.zeros(NBLK * C * P, dtype=np.float32)
    nrm_flat[col] = nrm_s

    # an all-invalid gather group breaks the DMA ucode/interp: give empty
    # groups one dummy valid index (row 0, zero weight via norm=0 pad)
    idx2 = idx_flat.reshape(NBLK, C * P)
    for b in np.nonzero(cnt_lo == 0)[0]:
        idx2[b, 0] = 0
    for b in np.nonzero((cnt - cnt_lo) == 0)[0]:
        idx2[b, CLO * P] = 0

    # wrap gather indices: idx i of a group sits at [i % 16, i // 16],
    # replicated over all 128 partitions.
    iw = idx_flat.reshape(NBLK, C * P)
    lo = iw[:, : CLO * P].reshape(NBLK, CLO * 8, 16).transpose(0, 2, 1)
    hi = iw[:, CLO * P :].reshape(NBLK, CHI * 8, 16).transpose(0, 2, 1)
    wrapped = np.concatenate([lo, hi], axis=2)            # [NBLK, 16, C*8]
    wrapped = np.tile(wrapped, (1, 8, 1))                 # [NBLK, 128, C*8]

    # per-chunk per-partition layouts
    dstp = dst_flat.reshape(NBLK, C, P).transpose(0, 2, 1)  # [NBLK, P, C]
    nrmp = nrm_flat.reshape(NBLK, C, P).transpose(0, 2, 1)

    eidx_cores, edst_cores, enrm_cores = [], [], []
    for c in range(NC):
        sl = slice(c * BPC, (c + 1) * BPC)
        eidx_cores.append(
            np.ascontiguousarray(
                wrapped[sl].transpose(1, 0, 2).reshape(P, BPC * C * 8)
            )
        )
        edst_cores.append(
            np.ascontiguousarray(dstp[sl].transpose(1, 0, 2).reshape(P, BPC * C))
        )
        enrm_cores.append(
            np.ascontiguousarray(nrmp[sl].transpose(1, 0, 2).reshape(P, BPC * C))
        )

    ztab = np.zeros((NPAD, FT), dtype=np.float16)
    ztab[:N, :F0] = z.astype(np.float16)

    w1p = np.zeros((FT, F1), dtype=np.float16)
    w1p[:F0] = W1.astype(np.float16)
    w2p = W2.astype(np.float16)

    b1bc = np.ascontiguousarray(np.broadcast_to(b1.astype(np.float32), (P, F1)))
    b2bc = np.ascontiguousarray(np.broadcast_to(b2.astype(np.float32), (P, F2)))
    iota = np.ascontiguousarray(np.broadcast_to(np.arange(P, dtype=np.float16), (P, P)))

    cnt_hi = cnt - cnt_lo
    cnts = np.empty((NBLK, 2), dtype=np.int32)
    cnts[:, 0] = np.maximum(cnt_lo, 1)
    cnts[:, 1] = np.maximum(cnt_hi, 1)
    cnt_cores = [
        np.ascontiguousarray(
            np.broadcast_to(
                cnts[c * BPC : (c + 1) * BPC].reshape(1, BPC * 2), (P, BPC * 2)
            )
        )
        for c in range(NC)
    ]

    edge = {
        "CLO": CLO,
        "CHI": CHI,
        "cnt": cnt_cores,
        "eidx": eidx_cores,
        "edst": edst_cores,
        "enrm": enrm_cores,
        "iot": iota,
    }
    return edge, ztab, w1p, b1bc, w2p, b2bc


def _run_layer(edge, xtab, wmat, bias, fout, out_f32):
    key = (edge["CLO"], edge["CHI"], fout, out_f32)
    if key not in _cache:
        _cache[key] = _build(edge["CLO"], edge["CHI"], fout, out_f32)
    nc = _cache[key]
    in_maps = [
        {
            "xtab": xtab,
            "eidx": edge["eidx"][c],
            "edst": edge["edst"][c],
            "enrm": edge["enrm"][c],
            "w": wmat,
            "bb": bias,
            "iot": edge["iot"],
            "cnt": edge["cnt"][c],
        }
        for c in range(NC)
    ]
    res = run_bass_kernel_spmd(nc, in_maps, core_ids=list(range(NC)))
    return np.concatenate([res.results[c]["out"] for c in range(NC)], axis=0)


# ---------------------------------------------------------------- entry
def kernel(z, edge_index, W1, b1, W2, b2):
    edge, ztab, w1p, b1bc, w2p, b2bc = _preprocess(
        np.asarray(z, dtype=np.float32),
        np.asarray(edge_index),
        np.asarray(W1, dtype=np.float32),
        np.asarray(b1, dtype=np.float32),
        np.asarray(W2, dtype=np.float32),
        np.asarray(b2, dtype=np.float32),
    )
    h1 = _run_layer(edge, ztab, w1p, b1bc, F1, out_f32=False)   # [NPAD, 128] fp16
    x_hat = _run_layer(edge, np.ascontiguousarray(h1), w2p, b2bc, F2, out_f32=True)
    return np.ascontiguousarray(x_hat[:N]).astype(np.float32)
